# revision 9
# baseline (speedup 1.0000x reference)
"""Trainium2 Bass kernel for nn_DynamicSparseConv.

Model (per sample):
    y  = mean(x, HW)                        [C]
    h  = gelu(y @ w1.T)                     [MID]
    w  = softmax((h @ w2.T).reshape(C, 9))  per-channel 3x3 kernels
    out = depthwise3x3(x, w) + x

Sharding: pure data parallel, batch 32 -> 4 samples on each of 8 cores.

Per-core design (per (sample b, channel-block cb of 128)):
  - x tile kept resident in SBUF as [128, 66, 64] (rows padded with zeros),
    read once from HBM, output written once: memory roofline ~32MiB/core.
  - channel means via DVE reduce over the interior view.
  - tiny MLP on PE (K=128 / K=32 matmuls, N=1), gelu via tanh formula
    (h_pre is in [-0.04, 0.04] so tanh-gelu == erf-gelu to ~1e-9; tanh and
    exp live in the same ACT table set, avoiding table swaps).
  - softmax over 9 taps: ACT exp with accum_out denominator, DVE reciprocal.
  - conv: 9 taps as diagonal-weighted matmuls on PE accumulating in PSUM.
    The x tile is stored flat as [128, 1 + 66*64 + 1]: rows padded with zero
    rows, plus one zero element on each end.  Every tap (r, s) is then a
    full-width contiguous [128, 512] matmul at flat offset r*64 + s; the
    horizontal wrap-around contributions (out col 0 picking up the previous
    row's col 63 for s=-1 taps, etc.) are subtracted afterwards with a few
    small strided DVE ops on the two edge columns.  Residual (+x) is fused
    into the DVE PSUM->SBUF merge.
"""

import numpy as np
from contextlib import ExitStack

import concourse.bass as bass
import concourse.tile as tile
from concourse import mybir
from concourse._compat import with_exitstack
from concourse.masks import make_identity
from concourse.bass_utils import run_bass_kernel_spmd

F32 = mybir.dt.float32
AL = mybir.AluOpType
AF = mybir.ActivationFunctionType

B, C, H, W = 32, 256, 64, 64
MID = 32
NCORES = 8
BPC = B // NCORES          # samples per core
P = 128
CB = C // P                # channel blocks
HP = H + 2                 # padded rows
FREE = H * W               # 4096
NCHUNK = 8                 # PSUM chunks per tile (8 output rows each)
RPC = H // NCHUNK          # rows per chunk

# tap order: full-coverage tap (0,0) first (it initializes every PSUM element
# of the accumulation group with start=True)
TAPS = [(0, 0), (-1, 0), (1, 0), (0, -1), (0, 1), (-1, -1), (-1, 1), (1, -1), (1, 1)]

SQRT_2_OVER_PI = 0.7978845608028654
GELU_C = 0.044715


@with_exitstack
def _build_body(ctx: ExitStack, tc: "tile.TileContext", x, w1t, w2r, out):
    nc = tc.nc

    consts = ctx.enter_context(tc.tile_pool(name="consts", bufs=1))
    xpool = ctx.enter_context(tc.tile_pool(name="xpool", bufs=4))
    opool = ctx.enter_context(tc.tile_pool(name="opool", bufs=2))
    mpool = ctx.enter_context(tc.tile_pool(name="mpool", bufs=4))
    dpool = ctx.enter_context(tc.tile_pool(name="dpool", bufs=2 * len(TAPS)))
    cpsum = ctx.enter_context(tc.tile_pool(name="cpsum", bufs=6, space="PSUM"))
    spsum = ctx.enter_context(tc.tile_pool(name="spsum", bufs=2, space="PSUM"))

    ident = consts.tile([P, P], F32)
    make_identity(nc, ident)

    w1t_sb = []
    for cb in range(CB):
        w1t_t = consts.tile([P, MID], F32, name=f"w1t_sb{cb}")
        nc.sync.dma_start(out=w1t_t, in_=w1t[cb * P:(cb + 1) * P, :])
        w1t_sb.append(w1t_t)

    w2r_sb = {}
    for cb in range(CB):
        for t in range(9):
            w2r_t = consts.tile([MID, P], F32, name=f"w2r_sb{cb}_{t}")
            nc.sync.dma_start(out=w2r_t, in_=w2r[cb, t])
            w2r_sb[(cb, t)] = w2r_t

    # flat padded x tile: [0 pad elem][row -1 zeros][64 x rows][row 64 zeros][0 pad elem]
    # logical padded row r (r in [-1, 64]) starts at flat offset 1 + (r+1)*64
    XF = 2 + HP * W            # 4226
    INT0 = 1 + W               # interior (x row 0 col 0) flat offset = 65

    for b in range(BPC):
        # ---- load both channel blocks of sample b, compute channel sums ----
        xts = []
        sums = mpool.tile([P, CB], F32, name=f"sums{b}", tag="sums")
        for cb in range(CB):
            xt = xpool.tile([P, XF], F32, name=f"xt{b}_{cb}", tag="xt")
            nc.gpsimd.memset(xt[:, 0:INT0], 0.0)
            nc.gpsimd.memset(xt[:, INT0 + FREE:XF], 0.0)
            nc.sync.dma_start(
                out=xt[:, INT0:INT0 + FREE],
                in_=x[b, cb * P:(cb + 1) * P].rearrange("c h w -> c (h w)"),
            )
            nc.vector.reduce_sum(
                out=sums[:, cb:cb + 1], in_=xt[:, INT0:INT0 + FREE],
                axis=mybir.AxisListType.X,
            )
            xts.append(xt)

        # ---- tiny MLP: h = gelu((sums/HW) @ w1.T) --------------------------
        hps = spsum.tile([P, 9], F32, name=f"hps{b}", tag="sps")
        for cb in range(CB):
            nc.tensor.matmul(
                hps[:MID, 0:1], lhsT=w1t_sb[cb], rhs=sums[:, cb:cb + 1],
                start=(cb == 0), stop=(cb == CB - 1),
            )
        # u = h_pre / (H*W)   (ACT copy with scale, PSUM -> SBUF)
        u = mpool.tile([MID, 1], F32, name=f"u{b}", tag="u")
        nc.scalar.mul(u, hps[:MID, 0:1], 1.0 / FREE)
        # tanh-based gelu: g = 0.5*u*(1+tanh(sqrt(2/pi)*(u + 0.044715 u^3)))
        # (the 0.5 is folded into w2r on the host)
        sq = mpool.tile([MID, 1], F32, name=f"sq{b}", tag="sq")
        nc.vector.tensor_mul(sq, u, u)
        c1 = mpool.tile([MID, 1], F32, name=f"c1{b}", tag="c1")
        nc.vector.tensor_scalar(
            out=c1, in0=sq, scalar1=GELU_C, scalar2=1.0, op0=AL.mult, op1=AL.add,
        )
        arg = mpool.tile([MID, 1], F32, name=f"arg{b}", tag="arg")
        nc.vector.tensor_mul(arg, u, c1)
        th = mpool.tile([MID, 1], F32, name=f"th{b}", tag="th")
        nc.scalar.activation(th, arg, AF.Tanh, scale=SQRT_2_OVER_PI)
        t1 = mpool.tile([MID, 1], F32, name=f"t1{b}", tag="t1")
        nc.vector.tensor_scalar_add(t1, th, 1.0)
        g = mpool.tile([MID, 1], F32, name=f"g{b}", tag="g")
        nc.vector.tensor_mul(g, u, t1)

        for cb in range(CB):
            # ---- per-channel tap weights: softmax(h @ w2.T) ----------------
            wg = spsum.tile([P, 9], F32, name=f"wg{b}_{cb}", tag="sps")
            for t in range(9):
                nc.tensor.matmul(
                    wg[:, t:t + 1], lhsT=w2r_sb[(cb, t)], rhs=g,
                    start=True, stop=True,
                )
            ew = mpool.tile([P, 9], F32, name=f"ew{b}_{cb}", tag="ew")
            den = mpool.tile([P, 1], F32, name=f"den{b}_{cb}", tag="den")
            nc.scalar.activation(ew, wg, AF.Exp, accum_out=den)
            rden = mpool.tile([P, 1], F32, name=f"rden{b}_{cb}", tag="rden")
            nc.vector.reciprocal(rden, den)
            smw = mpool.tile([P, 9], F32, name=f"smw{b}_{cb}", tag="smw")
            nc.vector.tensor_scalar_mul(smw, ew, rden)

            # ---- diagonal weight matrices for the 9 taps -------------------
            diags = {}
            for (r, s) in TAPS:
                tcol = (r + 1) * 3 + (s + 1)
                dg = dpool.tile([P, P], F32, name=f"dg{b}_{cb}_{tcol}", tag="dg")
                nc.vector.tensor_scalar_mul(dg, ident, smw[:, tcol:tcol + 1])
                diags[(r, s)] = dg

            # ---- depthwise conv: 9 diag matmuls per chunk into PSUM --------
            xt = xts[cb]
            CH = RPC * W  # 512 elements per chunk
            ot = opool.tile([P, FREE], F32, name=f"ot{b}_{cb}", tag="ot")
            for q in range(NCHUNK):
                ps = cpsum.tile([P, CH], F32, name=f"ps{b}_{cb}_{q}", tag="ps")
                for i, (r, s) in enumerate(TAPS):
                    off = INT0 + q * CH + r * W + s
                    nc.tensor.matmul(
                        ps,
                        lhsT=diags[(r, s)],
                        rhs=xt[:, off:off + CH],
                        start=(i == 0), stop=(i == len(TAPS) - 1),
                    )
                # merge + residual: out = psum + x
                nc.vector.tensor_add(
                    out=ot[:, q * CH:(q + 1) * CH],
                    in0=ps,
                    in1=xt[:, INT0 + q * CH:INT0 + (q + 1) * CH],
                )

            # ---- subtract the horizontal wrap-around garbage on the two
            # ---- edge columns (strided [128, 64] views, DVE)
            otr = ot.rearrange("p (h w) -> p h w", w=W)
            # out col 0: s=-1 taps read x[c, i-1+r, 63] at flat 64*(i+r+1)
            # out col 63: s=+1 taps read x[c, i+1+r, 0] at flat 64*(i+r+2)+1
            for col, s, base in ((0, -1, lambda r: W * (r + 1)),
                                 (W - 1, 1, lambda r: W * (r + 2) + 1)):
                tmp = mpool.tile([P, H], F32, name=f"ec{b}_{cb}_{col}", tag="ec")
                for k, r in enumerate((-1, 0, 1)):
                    tcol = (r + 1) * 3 + (s + 1)
                    # [128, 64] view of xt at flat offsets base(r) + 64*i
                    xv = bass.AP(
                        tensor=xt.tensor,
                        offset=xt.offset + base(r),
                        ap=[list(xt.ap[0]), [W, H]],
                    )
                    if k == 0:
                        nc.vector.tensor_scalar_mul(tmp, xv, smw[:, tcol:tcol + 1])
                    else:
                        nc.vector.scalar_tensor_tensor(
                            tmp, xv, smw[:, tcol:tcol + 1], tmp,
                            op0=AL.mult, op1=AL.add,
                        )
                nc.vector.tensor_sub(otr[:, :, col], otr[:, :, col], tmp)

            nc.sync.dma_start(
                out=out[b, cb * P:(cb + 1) * P].rearrange("c h w -> c (h w)"),
                in_=ot,
            )


def build_nc():
    nc = bass.Bass(trn_type="TRN2")
    x = nc.dram_tensor("x", [BPC, C, H, W], F32, kind="ExternalInput")
    w1t = nc.dram_tensor("w1t", [C, MID], F32, kind="ExternalInput")
    w2r = nc.dram_tensor("w2r", [CB, 9, MID, P], F32, kind="ExternalInput")
    out = nc.dram_tensor("out", [BPC, C, H, W], F32, kind="ExternalOutput")
    with tile.TileContext(nc) as tc:
        _build_body(tc, x, w1t, w2r, out)
    return nc


def host_prep(w1: np.ndarray, w2: np.ndarray):
    """Layout-only prep of the (tiny) shared weights."""
    w1t = np.ascontiguousarray(np.asarray(w1, dtype=np.float32).T)  # [C, MID]
    # w2 rows are r = c*9 + t ; -> [cb, t, mid, c_local], pre-scaled by 0.5
    # (folds the 0.5 of gelu: g_kernel = u*(1+tanh(...)) = 2*gelu(u))
    w2r = np.asarray(w2, dtype=np.float32).reshape(CB, P, 9, MID)
    w2r = np.ascontiguousarray(w2r.transpose(0, 2, 3, 1)) * 0.5
    return w1t, w2r


# TPB instructions have a single EVENTS (wait) slot and this walrus refuses
# >1 sync-wait on them (Matmult, TensorScalarPtr, DMACopy, ...).  Drain is
# Tile's standard multi-wait tail barrier, which walrus does handle.
_SPLIT_WAIT_SKIP = {"EventSemaphore"}


def _split_matmul_waits_json(data: bytes) -> bytes:
    """Move excess sync-waits on single-wait-slot instructions onto
    EventSemaphore instructions inserted immediately before them on the same
    engine queue (semantically identical)."""
    import orjson

    m = orjson.loads(data)
    cnt = 0
    for fn in m.get("functions", []):
        for bb in fn.get("blocks", []):
            insts = bb.get("instructions")
            if not insts:
                continue
            out = []
            changed = False
            for ins in insts:
                si = ins.get("sync_info")
                if (
                    ins.get("opcode") not in _SPLIT_WAIT_SKIP
                    and si
                    and len(si.get("on_wait") or []) > 1
                ):
                    waits = si["on_wait"]
                    for w in waits[:-1]:
                        out.append({
                            "name": f"EVW-{cnt}",
                            "opcode": "EventSemaphore",
                            "engine": ins["engine"],
                            "ins": [],
                            "outs": [],
                            "debug": ins.get("debug", 0),
                            "sync_info": {"on_wait": [w], "on_update": []},
                        })
                        cnt += 1
                    si["on_wait"] = [waits[-1]]
                    changed = True
                out.append(ins)
            if changed:
                bb["instructions"] = out
    return orjson.dumps(m)


_CACHE: dict = {}


def _get_nc():
    if "nc" not in _CACHE:
        nc = build_nc()
        orig = nc.to_json_bytes
        nc.to_json_bytes = lambda: _split_matmul_waits_json(orig())
        _CACHE["nc"] = nc
    return _CACHE["nc"]


def kernel(x, w1, w2, trace: bool = False, **run_kwargs):
    x = np.ascontiguousarray(np.asarray(x, dtype=np.float32))
    assert x.shape == (B, C, H, W)
    w1t, w2r = host_prep(w1, w2)

    nc = _get_nc()
    in_maps = [
        {"x": x[i * BPC:(i + 1) * BPC], "w1t": w1t, "w2r": w2r}
        for i in range(NCORES)
    ]
    res = run_bass_kernel_spmd(
        nc, in_maps, core_ids=list(range(NCORES)), trace=trace, **run_kwargs
    )
    _CACHE["last_results"] = res
    out = np.concatenate([res.results[i]["out"] for i in range(NCORES)], axis=0)
    return out


# revision 18
# speedup vs baseline: 3.0547x; 3.0547x over previous
"""Trainium2 Bass kernel for nn_DynamicSparseConv.

Model (per sample):
    y  = mean(x, HW)                        [C]
    h  = gelu(y @ w1.T)                     [MID]
    w  = softmax((h @ w2.T).reshape(C, 9))  per-channel 3x3 kernels
    out = depthwise3x3(x, w) + x

Sharding: pure data parallel, batch 32 -> 4 samples on each of 8 cores.

Per-core design (per (sample b, channel-block cb of 128)):
  - x tile kept resident in SBUF as [128, 66, 64] (rows padded with zeros),
    read once from HBM, output written once: memory roofline ~32MiB/core.
  - channel means via DVE reduce over the interior view.
  - tiny MLP on PE (K=128 / K=32 matmuls, N=1), gelu via tanh formula
    (h_pre is in [-0.04, 0.04] so tanh-gelu == erf-gelu to ~1e-9; tanh and
    exp live in the same ACT table set, avoiding table swaps).
  - softmax over 9 taps: ACT exp with accum_out denominator, DVE reciprocal.
  - conv: 9 taps as diagonal-weighted matmuls on PE accumulating in PSUM.
    The x tile is stored flat as [128, 1 + 66*64 + 1]: rows padded with zero
    rows, plus one zero element on each end.  Every tap (r, s) is then a
    full-width contiguous [128, 512] matmul at flat offset r*64 + s; the
    horizontal wrap-around contributions (out col 0 picking up the previous
    row's col 63 for s=-1 taps, etc.) are subtracted afterwards with a few
    small strided DVE ops on the two edge columns.  Residual (+x) is fused
    into the DVE PSUM->SBUF merge.
"""

import numpy as np
from contextlib import ExitStack

import concourse.bass as bass
import concourse.tile as tile
from concourse import mybir
from concourse._compat import with_exitstack
from concourse.masks import make_identity
from concourse.bass_utils import run_bass_kernel_spmd

F32 = mybir.dt.float32
BF16 = mybir.dt.bfloat16
AL = mybir.AluOpType
AF = mybir.ActivationFunctionType

B, C, H, W = 32, 256, 64, 64
MID = 32
NCORES = 8
BPC = B // NCORES          # samples per core
P = 128
CB = C // P                # channel blocks
HP = H + 2                 # padded rows
FREE = H * W               # 4096
NCHUNK = 8                 # PSUM chunks per tile (8 output rows each)
RPC = H // NCHUNK          # rows per chunk

# tap order: full-coverage tap (0,0) first (it initializes every PSUM element
# of the accumulation group with start=True)
TAPS = [(0, 0), (-1, 0), (1, 0), (0, -1), (0, 1), (-1, -1), (-1, 1), (1, -1), (1, 1)]

SQRT_2_OVER_PI = 0.7978845608028654
GELU_C = 0.044715


@with_exitstack
def _build_body(ctx: ExitStack, tc: "tile.TileContext", x, w1t, w2r, out):
    nc = tc.nc

    consts = ctx.enter_context(tc.tile_pool(name="consts", bufs=1))
    xpool = ctx.enter_context(tc.tile_pool(name="xpool", bufs=4))
    xbpool = ctx.enter_context(tc.tile_pool(name="xbpool", bufs=4))
    opool = ctx.enter_context(tc.tile_pool(name="opool", bufs=2))
    mpool = ctx.enter_context(tc.tile_pool(name="mpool", bufs=4))
    dpool = ctx.enter_context(tc.tile_pool(name="dpool", bufs=2 * len(TAPS)))
    cpsum = ctx.enter_context(tc.tile_pool(name="cpsum", bufs=6, space="PSUM"))
    spsum = ctx.enter_context(tc.tile_pool(name="spsum", bufs=2, space="PSUM"))

    ident = consts.tile([P, P], F32)
    make_identity(nc, ident)

    w1t_sb = []
    for cb in range(CB):
        w1t_t = consts.tile([P, MID], BF16, name=f"w1t_sb{cb}")
        nc.sync.dma_start(out=w1t_t, in_=w1t[cb * P:(cb + 1) * P, :])
        w1t_sb.append(w1t_t)

    w2r_sb = {}
    for cb in range(CB):
        for t in range(9):
            w2r_t = consts.tile([MID, P], BF16, name=f"w2r_sb{cb}_{t}")
            nc.sync.dma_start(out=w2r_t, in_=w2r[cb, t])
            w2r_sb[(cb, t)] = w2r_t

    # bf16 conv-input tile, flat padded layout:
    # [0 pad elem][row -1 zeros][64 x rows][row 64 zeros][0 pad elem];
    # logical padded row r (r in [-1, 64]) starts at flat offset 1 + (r+1)*64.
    # The f32 x tile stays plain [P, 4096] (residual + exactness of means).
    XF = 2 + HP * W            # 4226
    INT0 = 1 + W               # interior (x row 0 col 0) flat offset = 65

    for b in range(BPC):
        # ---- load both channel blocks of sample b ----
        # ACT pass does the f32->bf16 cast AND the channel sums (accum_out)
        xts = []
        xbs = []
        sums = mpool.tile([P, CB], F32, name=f"sums{b}", tag="sums")
        for cb in range(CB):
            xt = xpool.tile([P, FREE], F32, name=f"xt{b}_{cb}", tag="xt")
            nc.sync.dma_start(
                out=xt,
                in_=x[b, cb * P:(cb + 1) * P].rearrange("c h w -> c (h w)"),
            )
            xb = xbpool.tile([P, XF], BF16, name=f"xb{b}_{cb}", tag="xb")
            nc.gpsimd.memset(xb[:, 0:INT0], 0.0)
            nc.gpsimd.memset(xb[:, INT0 + FREE:XF], 0.0)
            nc.scalar.activation(
                out=xb[:, INT0:INT0 + FREE], in_=xt, func=AF.Copy,
                accum_out=sums[:, cb:cb + 1],
            )
            xts.append(xt)
            xbs.append(xb)

        # ---- tiny MLP: h = gelu((sums/HW) @ w1.T) --------------------------
        sums_bf = mpool.tile([P, CB], BF16, name=f"sums_bf{b}", tag="sums_bf")
        nc.vector.tensor_copy(sums_bf, sums)
        hps = spsum.tile([P, 9], F32, name=f"hps{b}", tag="sps")
        for cb in range(CB):
            nc.tensor.matmul(
                hps[:MID, 0:1], lhsT=w1t_sb[cb], rhs=sums_bf[:, cb:cb + 1],
                start=(cb == 0), stop=(cb == CB - 1),
            )
        # u = h_pre / (H*W)   (ACT copy with scale, PSUM -> SBUF)
        u = mpool.tile([MID, 1], F32, name=f"u{b}", tag="u")
        nc.scalar.mul(u, hps[:MID, 0:1], 1.0 / FREE)
        # tanh-based gelu: g = 0.5*u*(1+tanh(sqrt(2/pi)*(u + 0.044715 u^3)))
        # (the 0.5 is folded into w2r on the host)
        sq = mpool.tile([MID, 1], F32, name=f"sq{b}", tag="sq")
        nc.vector.tensor_mul(sq, u, u)
        c1 = mpool.tile([MID, 1], F32, name=f"c1{b}", tag="c1")
        nc.vector.tensor_scalar(
            out=c1, in0=sq, scalar1=GELU_C, scalar2=1.0, op0=AL.mult, op1=AL.add,
        )
        arg = mpool.tile([MID, 1], F32, name=f"arg{b}", tag="arg")
        nc.vector.tensor_mul(arg, u, c1)
        th = mpool.tile([MID, 1], F32, name=f"th{b}", tag="th")
        nc.scalar.activation(th, arg, AF.Tanh, scale=SQRT_2_OVER_PI)
        t1 = mpool.tile([MID, 1], F32, name=f"t1{b}", tag="t1")
        nc.vector.tensor_scalar_add(t1, th, 1.0)
        g = mpool.tile([MID, 1], BF16, name=f"g{b}", tag="g")
        nc.vector.tensor_mul(g, u, t1)

        for cb in range(CB):
            # ---- per-channel tap weights: softmax(h @ w2.T) ----------------
            wg = spsum.tile([P, 9], F32, name=f"wg{b}_{cb}", tag="sps")
            for t in range(9):
                nc.tensor.matmul(
                    wg[:, t:t + 1], lhsT=w2r_sb[(cb, t)], rhs=g,
                    start=True, stop=True,
                )
            ew = mpool.tile([P, 9], F32, name=f"ew{b}_{cb}", tag="ew")
            den = mpool.tile([P, 1], F32, name=f"den{b}_{cb}", tag="den")
            nc.scalar.activation(ew, wg, AF.Exp, accum_out=den)
            rden = mpool.tile([P, 1], F32, name=f"rden{b}_{cb}", tag="rden")
            nc.vector.reciprocal(rden, den)
            smw = mpool.tile([P, 9], F32, name=f"smw{b}_{cb}", tag="smw")
            nc.vector.tensor_scalar_mul(smw, ew, rden)

            # ---- diagonal weight matrices for the 9 taps -------------------
            diags = {}
            for (r, s) in TAPS:
                tcol = (r + 1) * 3 + (s + 1)
                dg = dpool.tile([P, P], BF16, name=f"dg{b}_{cb}_{tcol}", tag="dg")
                nc.vector.tensor_scalar_mul(dg, ident, smw[:, tcol:tcol + 1])
                diags[(r, s)] = dg

            # ---- depthwise conv: 9 diag matmuls per chunk into PSUM --------
            xt = xts[cb]
            xb = xbs[cb]
            CH = RPC * W  # 512 elements per chunk
            ot = opool.tile([P, FREE], F32, name=f"ot{b}_{cb}", tag="ot")
            for q in range(NCHUNK):
                ps = cpsum.tile([P, CH], F32, name=f"ps{b}_{cb}_{q}", tag="ps")
                for i, (r, s) in enumerate(TAPS):
                    off = INT0 + q * CH + r * W + s
                    nc.tensor.matmul(
                        ps,
                        lhsT=diags[(r, s)],
                        rhs=xb[:, off:off + CH],
                        start=(i == 0), stop=(i == len(TAPS) - 1),
                    )
                # merge + residual: out = psum + x (f32)
                nc.vector.tensor_add(
                    out=ot[:, q * CH:(q + 1) * CH],
                    in0=ps,
                    in1=xt[:, q * CH:(q + 1) * CH],
                )

            # ---- subtract the horizontal wrap-around garbage on the two
            # ---- edge columns (strided [128, 64] views, DVE)
            otr = ot.rearrange("p (h w) -> p h w", w=W)
            # out col 0: s=-1 taps read x[c, i-1+r, 63] at flat 64*(i+r+1)
            # out col 63: s=+1 taps read x[c, i+1+r, 0] at flat 64*(i+r+2)+1
            for col, s, base in ((0, -1, lambda r: W * (r + 1)),
                                 (W - 1, 1, lambda r: W * (r + 2) + 1)):
                tmp = mpool.tile([P, H], F32, name=f"ec{b}_{cb}_{col}", tag="ec")
                for k, r in enumerate((-1, 0, 1)):
                    tcol = (r + 1) * 3 + (s + 1)
                    # [128, 64] view of xb at flat offsets base(r) + 64*i
                    xv = bass.AP(
                        tensor=xb.tensor,
                        offset=xb.offset + base(r),
                        ap=[list(xb.ap[0]), [W, H]],
                    )
                    if k == 0:
                        nc.vector.tensor_scalar_mul(tmp, xv, smw[:, tcol:tcol + 1])
                    else:
                        nc.vector.scalar_tensor_tensor(
                            tmp, xv, smw[:, tcol:tcol + 1], tmp,
                            op0=AL.mult, op1=AL.add,
                        )
                nc.vector.tensor_sub(otr[:, :, col], otr[:, :, col], tmp)

            nc.sync.dma_start(
                out=out[b, cb * P:(cb + 1) * P].rearrange("c h w -> c (h w)"),
                in_=ot,
            )


def build_nc():
    nc = bass.Bass(trn_type="TRN2")
    x = nc.dram_tensor("x", [BPC, C, H, W], F32, kind="ExternalInput")
    w1t = nc.dram_tensor("w1t", [C, MID], BF16, kind="ExternalInput")
    w2r = nc.dram_tensor("w2r", [CB, 9, MID, P], BF16, kind="ExternalInput")
    out = nc.dram_tensor("out", [BPC, C, H, W], F32, kind="ExternalOutput")
    with tile.TileContext(nc) as tc:
        _build_body(tc, x, w1t, w2r, out)
    return nc


def host_prep(w1: np.ndarray, w2: np.ndarray):
    """Layout/dtype-only prep of the (tiny) shared weights."""
    import ml_dtypes

    w1t = np.ascontiguousarray(np.asarray(w1, dtype=np.float32).T)  # [C, MID]
    # w2 rows are r = c*9 + t ; -> [cb, t, mid, c_local], pre-scaled by 0.5
    # (folds the 0.5 of gelu: g_kernel = u*(1+tanh(...)) = 2*gelu(u))
    w2r = np.asarray(w2, dtype=np.float32).reshape(CB, P, 9, MID)
    w2r = np.ascontiguousarray(w2r.transpose(0, 2, 3, 1)) * 0.5
    return w1t.astype(ml_dtypes.bfloat16), w2r.astype(ml_dtypes.bfloat16)


# TPB instructions have a single EVENTS (wait) slot and this walrus refuses
# >1 sync-wait on them (Matmult, TensorScalarPtr, DMACopy, ...).  Drain is
# Tile's standard multi-wait tail barrier, which walrus does handle.
_SPLIT_WAIT_SKIP = {"EventSemaphore"}


def _split_matmul_waits_json(data: bytes) -> bytes:
    """Move excess sync-waits on single-wait-slot instructions onto
    EventSemaphore instructions inserted immediately before them on the same
    engine queue (semantically identical)."""
    import orjson

    m = orjson.loads(data)
    cnt = 0
    for fn in m.get("functions", []):
        for bb in fn.get("blocks", []):
            insts = bb.get("instructions")
            if not insts:
                continue
            out = []
            changed = False
            for ins in insts:
                si = ins.get("sync_info")
                if (
                    ins.get("opcode") not in _SPLIT_WAIT_SKIP
                    and si
                    and len(si.get("on_wait") or []) > 1
                ):
                    waits = si["on_wait"]
                    for w in waits[:-1]:
                        out.append({
                            "name": f"EVW-{cnt}",
                            "opcode": "EventSemaphore",
                            "engine": ins["engine"],
                            "ins": [],
                            "outs": [],
                            "debug": ins.get("debug", 0),
                            "sync_info": {"on_wait": [w], "on_update": []},
                        })
                        cnt += 1
                    si["on_wait"] = [waits[-1]]
                    changed = True
                out.append(ins)
            if changed:
                bb["instructions"] = out
    return orjson.dumps(m)


_CACHE: dict = {}


def _get_nc():
    if "nc" not in _CACHE:
        nc = build_nc()
        orig = nc.to_json_bytes
        nc.to_json_bytes = lambda: _split_matmul_waits_json(orig())
        _CACHE["nc"] = nc
    return _CACHE["nc"]


def kernel(x, w1, w2, trace: bool = False, **run_kwargs):
    x = np.ascontiguousarray(np.asarray(x, dtype=np.float32))
    assert x.shape == (B, C, H, W)
    w1t, w2r = host_prep(w1, w2)

    nc = _get_nc()
    in_maps = [
        {"x": x[i * BPC:(i + 1) * BPC], "w1t": w1t, "w2r": w2r}
        for i in range(NCORES)
    ]
    res = run_bass_kernel_spmd(
        nc, in_maps, core_ids=list(range(NCORES)), trace=trace, **run_kwargs
    )
    _CACHE["last_results"] = res
    out = np.concatenate([res.results[i]["out"] for i in range(NCORES)], axis=0)
    return out


# revision 19
# speedup vs baseline: 3.2382x; 1.0601x over previous
"""Trainium2 Bass kernel for nn_DynamicSparseConv.

Model (per sample):
    y  = mean(x, HW)                        [C]
    h  = gelu(y @ w1.T)                     [MID]
    w  = softmax((h @ w2.T).reshape(C, 9))  per-channel 3x3 kernels
    out = depthwise3x3(x, w) + x

Sharding: pure data parallel, batch 32 -> 4 samples on each of 8 cores.

Per-core design (per (sample b, channel-block cb of 128)):
  - x tile kept resident in SBUF as [128, 66, 64] (rows padded with zeros),
    read once from HBM, output written once: memory roofline ~32MiB/core.
  - channel means via DVE reduce over the interior view.
  - tiny MLP on PE (K=128 / K=32 matmuls, N=1), gelu via tanh formula
    (h_pre is in [-0.04, 0.04] so tanh-gelu == erf-gelu to ~1e-9; tanh and
    exp live in the same ACT table set, avoiding table swaps).
  - softmax over 9 taps: ACT exp with accum_out denominator, DVE reciprocal.
  - conv: 9 taps as diagonal-weighted matmuls on PE accumulating in PSUM.
    The x tile is stored flat as [128, 1 + 66*64 + 1]: rows padded with zero
    rows, plus one zero element on each end.  Every tap (r, s) is then a
    full-width contiguous [128, 512] matmul at flat offset r*64 + s; the
    horizontal wrap-around contributions (out col 0 picking up the previous
    row's col 63 for s=-1 taps, etc.) are subtracted afterwards with a few
    small strided DVE ops on the two edge columns.  Residual (+x) is fused
    into the DVE PSUM->SBUF merge.
"""

import numpy as np
from contextlib import ExitStack

import concourse.bass as bass
import concourse.tile as tile
from concourse import mybir
from concourse._compat import with_exitstack
from concourse.masks import make_identity
from concourse.bass_utils import run_bass_kernel_spmd

F32 = mybir.dt.float32
BF16 = mybir.dt.bfloat16
AL = mybir.AluOpType
AF = mybir.ActivationFunctionType

B, C, H, W = 32, 256, 64, 64
MID = 32
NCORES = 8
BPC = B // NCORES          # samples per core
P = 128
CB = C // P                # channel blocks
HP = H + 2                 # padded rows
FREE = H * W               # 4096
NCHUNK = 8                 # PSUM chunks per tile (8 output rows each)
RPC = H // NCHUNK          # rows per chunk

# tap order: full-coverage tap (0,0) first (it initializes every PSUM element
# of the accumulation group with start=True)
TAPS = [(0, 0), (-1, 0), (1, 0), (0, -1), (0, 1), (-1, -1), (-1, 1), (1, -1), (1, 1)]

SQRT_2_OVER_PI = 0.7978845608028654
GELU_C = 0.044715


# bf16 conv-input tile, flat padded layout:
# [0 pad elem][row -1 zeros][64 x rows][row 64 zeros][0 pad elem];
# logical padded row r (r in [-1, 64]) starts at flat offset 1 + (r+1)*64.
# The f32 x tile stays plain [P, 4096] (residual + exactness of means).
XF = 2 + HP * W            # 4226
INT0 = 1 + W               # interior (x row 0 col 0) flat offset = 65
CH = RPC * W               # 512 elements per PSUM chunk


@with_exitstack
def _build_body(ctx: ExitStack, tc: "tile.TileContext", x, w1t, w2r, out):
    nc = tc.nc

    consts = ctx.enter_context(tc.tile_pool(name="consts", bufs=1))
    xpool = ctx.enter_context(tc.tile_pool(name="xpool", bufs=4))
    xbpool = ctx.enter_context(tc.tile_pool(name="xbpool", bufs=4))
    opool = ctx.enter_context(tc.tile_pool(name="opool", bufs=2))
    mpool = ctx.enter_context(tc.tile_pool(name="mpool", bufs=4))
    dpool = ctx.enter_context(tc.tile_pool(name="dpool", bufs=2 * len(TAPS)))
    cpsum = ctx.enter_context(tc.tile_pool(name="cpsum", bufs=6, space="PSUM"))
    spsum = ctx.enter_context(tc.tile_pool(name="spsum", bufs=2, space="PSUM"))

    st = {}  # per-sample pipeline state

    def load(b):
        """DMA in both channel blocks of sample b + bf16 cast + channel sums."""
        xts, xbs = [], []
        sums = mpool.tile([P, CB], F32, name=f"sums{b}", tag="sums")
        for cb in range(CB):
            xt = xpool.tile([P, FREE], F32, name=f"xt{b}_{cb}", tag="xt")
            nc.sync.dma_start(
                out=xt,
                in_=x[b, cb * P:(cb + 1) * P].rearrange("c h w -> c (h w)"),
            )
            xb = xbpool.tile([P, XF], BF16, name=f"xb{b}_{cb}", tag="xb")
            nc.gpsimd.memset(xb[:, 0:INT0], 0.0)
            nc.gpsimd.memset(xb[:, INT0 + FREE:XF], 0.0)
            # one ACT pass: f32->bf16 cast AND per-channel sums (accum_out)
            nc.scalar.activation(
                out=xb[:, INT0:INT0 + FREE], in_=xt, func=AF.Copy,
                accum_out=sums[:, cb:cb + 1],
            )
            xts.append(xt)
            xbs.append(xb)
        st[b] = {"xts": xts, "xbs": xbs, "sums": sums}

    def prep(b, weights):
        """MLP -> softmax tap weights -> diagonal matrices for sample b."""
        w1t_sb, w2r_sb, ident = weights
        sums = st[b]["sums"]
        sums_bf = mpool.tile([P, CB], BF16, name=f"sums_bf{b}", tag="sums_bf")
        nc.vector.tensor_copy(sums_bf, sums)
        hps = spsum.tile([P, 9], F32, name=f"hps{b}", tag="sps")
        for cb in range(CB):
            nc.tensor.matmul(
                hps[:MID, 0:1], lhsT=w1t_sb[:, cb, :], rhs=sums_bf[:, cb:cb + 1],
                start=(cb == 0), stop=(cb == CB - 1),
            )
        # u = h_pre / (H*W)   (ACT copy with scale, PSUM -> SBUF)
        u = mpool.tile([MID, 1], F32, name=f"u{b}", tag="u")
        nc.scalar.mul(u, hps[:MID, 0:1], 1.0 / FREE)
        # tanh-based gelu: g = 0.5*u*(1+tanh(sqrt(2/pi)*(u + 0.044715 u^3)))
        # (the 0.5 is folded into w2r on the host)
        sq = mpool.tile([MID, 1], F32, name=f"sq{b}", tag="sq")
        nc.vector.tensor_mul(sq, u, u)
        c1 = mpool.tile([MID, 1], F32, name=f"c1{b}", tag="c1")
        nc.vector.tensor_scalar(
            out=c1, in0=sq, scalar1=GELU_C, scalar2=1.0, op0=AL.mult, op1=AL.add,
        )
        arg = mpool.tile([MID, 1], F32, name=f"arg{b}", tag="arg")
        nc.vector.tensor_mul(arg, u, c1)
        th = mpool.tile([MID, 1], F32, name=f"th{b}", tag="th")
        nc.scalar.activation(th, arg, AF.Tanh, scale=SQRT_2_OVER_PI)
        t1 = mpool.tile([MID, 1], F32, name=f"t1{b}", tag="t1")
        nc.vector.tensor_scalar_add(t1, th, 1.0)
        g = mpool.tile([MID, 1], BF16, name=f"g{b}", tag="g")
        nc.vector.tensor_mul(g, u, t1)

        st[b]["smw"] = []
        st[b]["diags"] = []
        for cb in range(CB):
            wg = spsum.tile([P, 9], F32, name=f"wg{b}_{cb}", tag="sps")
            for t in range(9):
                nc.tensor.matmul(
                    wg[:, t:t + 1], lhsT=w2r_sb[:, cb * 9 + t, :], rhs=g,
                    start=True, stop=True,
                )
            ew = mpool.tile([P, 9], F32, name=f"ew{b}_{cb}", tag="ew")
            den = mpool.tile([P, 1], F32, name=f"den{b}_{cb}", tag="den")
            nc.scalar.activation(ew, wg, AF.Exp, accum_out=den)
            rden = mpool.tile([P, 1], F32, name=f"rden{b}_{cb}", tag="rden")
            nc.vector.reciprocal(rden, den)
            smw = mpool.tile([P, 9], F32, name=f"smw{b}_{cb}", tag="smw")
            nc.vector.tensor_scalar_mul(smw, ew, rden)

            diags = {}
            for (r, s) in TAPS:
                tcol = (r + 1) * 3 + (s + 1)
                dg = dpool.tile([P, P], BF16, name=f"dg{b}_{cb}_{tcol}", tag="dg")
                nc.vector.tensor_scalar_mul(dg, ident, smw[:, tcol:tcol + 1])
                diags[(r, s)] = dg
            st[b]["smw"].append(smw)
            st[b]["diags"].append(diags)

    def conv(b):
        """Depthwise conv + residual merge + edge fixups + output DMA."""
        for cb in range(CB):
            xt = st[b]["xts"][cb]
            xb = st[b]["xbs"][cb]
            smw = st[b]["smw"][cb]
            diags = st[b]["diags"][cb]
            ot = opool.tile([P, FREE], F32, name=f"ot{b}_{cb}", tag="ot")
            for q in range(NCHUNK):
                ps = cpsum.tile([P, CH], F32, name=f"ps{b}_{cb}_{q}", tag="ps")
                for i, (r, s) in enumerate(TAPS):
                    off = INT0 + q * CH + r * W + s
                    nc.tensor.matmul(
                        ps,
                        lhsT=diags[(r, s)],
                        rhs=xb[:, off:off + CH],
                        start=(i == 0), stop=(i == len(TAPS) - 1),
                    )
                # merge + residual: out = psum + x (f32)
                nc.vector.tensor_add(
                    out=ot[:, q * CH:(q + 1) * CH],
                    in0=ps,
                    in1=xt[:, q * CH:(q + 1) * CH],
                )

            # subtract the horizontal wrap-around garbage on the two edge
            # columns (strided [128, 64] views, DVE)
            otr = ot.rearrange("p (h w) -> p h w", w=W)
            # out col 0: s=-1 taps read x[c, i-1+r, 63] at flat 64*(i+r+1)
            # out col 63: s=+1 taps read x[c, i+1+r, 0] at flat 64*(i+r+2)+1
            for col, s, base in ((0, -1, lambda r: W * (r + 1)),
                                 (W - 1, 1, lambda r: W * (r + 2) + 1)):
                tmp = mpool.tile([P, H], F32, name=f"ec{b}_{cb}_{col}", tag="ec")
                for k, r in enumerate((-1, 0, 1)):
                    tcol = (r + 1) * 3 + (s + 1)
                    xv = bass.AP(
                        tensor=xb.tensor,
                        offset=xb.offset + base(r),
                        ap=[list(xb.ap[0]), [W, H]],
                    )
                    if k == 0:
                        nc.vector.tensor_scalar_mul(tmp, xv, smw[:, tcol:tcol + 1])
                    else:
                        nc.vector.scalar_tensor_tensor(
                            tmp, xv, smw[:, tcol:tcol + 1], tmp,
                            op0=AL.mult, op1=AL.add,
                        )
                nc.vector.tensor_sub(otr[:, :, col], otr[:, :, col], tmp)

            # output DMA on the scalar HWDGE queue (parallel issue with the
            # input DMAs on the sync queue)
            nc.scalar.dma_start(
                out=out[b, cb * P:(cb + 1) * P].rearrange("c h w -> c (h w)"),
                in_=ot,
            )
        del st[b]

    # ---- emission: sample 0's x DMAs first (they gate the pipeline) -------
    load(0)

    # weights: two fused DMAs + identity
    ident = consts.tile([P, P], F32)
    make_identity(nc, ident)
    w1t_sb = consts.tile([P, CB, MID], BF16)
    nc.sync.dma_start(out=w1t_sb, in_=w1t.rearrange("(cb c) m -> c cb m", cb=CB))
    w2r_sb = consts.tile([MID, CB * 9, P], BF16)
    nc.sync.dma_start(out=w2r_sb, in_=w2r.rearrange("cb t m c -> m (cb t) c"))
    weights = (w1t_sb, w2r_sb, ident)

    load(1)
    prep(0, weights)
    for b in range(BPC):
        if b + 2 < BPC:
            load(b + 2)
        if b + 1 < BPC:
            prep(b + 1, weights)
        conv(b)


def build_nc():
    nc = bass.Bass(trn_type="TRN2")
    x = nc.dram_tensor("x", [BPC, C, H, W], F32, kind="ExternalInput")
    w1t = nc.dram_tensor("w1t", [C, MID], BF16, kind="ExternalInput")
    w2r = nc.dram_tensor("w2r", [CB, 9, MID, P], BF16, kind="ExternalInput")
    out = nc.dram_tensor("out", [BPC, C, H, W], F32, kind="ExternalOutput")
    with tile.TileContext(nc) as tc:
        _build_body(tc, x, w1t, w2r, out)
    return nc


def host_prep(w1: np.ndarray, w2: np.ndarray):
    """Layout/dtype-only prep of the (tiny) shared weights."""
    import ml_dtypes

    w1t = np.ascontiguousarray(np.asarray(w1, dtype=np.float32).T)  # [C, MID]
    # w2 rows are r = c*9 + t ; -> [cb, t, mid, c_local], pre-scaled by 0.5
    # (folds the 0.5 of gelu: g_kernel = u*(1+tanh(...)) = 2*gelu(u))
    w2r = np.asarray(w2, dtype=np.float32).reshape(CB, P, 9, MID)
    w2r = np.ascontiguousarray(w2r.transpose(0, 2, 3, 1)) * 0.5
    return w1t.astype(ml_dtypes.bfloat16), w2r.astype(ml_dtypes.bfloat16)


# TPB instructions have a single EVENTS (wait) slot and this walrus refuses
# >1 sync-wait on them (Matmult, TensorScalarPtr, DMACopy, ...).  Drain is
# Tile's standard multi-wait tail barrier, which walrus does handle.
_SPLIT_WAIT_SKIP = {"EventSemaphore"}


def _split_matmul_waits_json(data: bytes) -> bytes:
    """Move excess sync-waits on single-wait-slot instructions onto
    EventSemaphore instructions inserted immediately before them on the same
    engine queue (semantically identical)."""
    import orjson

    m = orjson.loads(data)
    cnt = 0
    for fn in m.get("functions", []):
        for bb in fn.get("blocks", []):
            insts = bb.get("instructions")
            if not insts:
                continue
            out = []
            changed = False
            for ins in insts:
                si = ins.get("sync_info")
                if (
                    ins.get("opcode") not in _SPLIT_WAIT_SKIP
                    and si
                    and len(si.get("on_wait") or []) > 1
                ):
                    waits = si["on_wait"]
                    for w in waits[:-1]:
                        out.append({
                            "name": f"EVW-{cnt}",
                            "opcode": "EventSemaphore",
                            "engine": ins["engine"],
                            "ins": [],
                            "outs": [],
                            "debug": ins.get("debug", 0),
                            "sync_info": {"on_wait": [w], "on_update": []},
                        })
                        cnt += 1
                    si["on_wait"] = [waits[-1]]
                    changed = True
                out.append(ins)
            if changed:
                bb["instructions"] = out
    return orjson.dumps(m)


_CACHE: dict = {}


def _get_nc():
    if "nc" not in _CACHE:
        nc = build_nc()
        orig = nc.to_json_bytes
        nc.to_json_bytes = lambda: _split_matmul_waits_json(orig())
        _CACHE["nc"] = nc
    return _CACHE["nc"]


def kernel(x, w1, w2, trace: bool = False, **run_kwargs):
    x = np.ascontiguousarray(np.asarray(x, dtype=np.float32))
    assert x.shape == (B, C, H, W)
    w1t, w2r = host_prep(w1, w2)

    nc = _get_nc()
    in_maps = [
        {"x": x[i * BPC:(i + 1) * BPC], "w1t": w1t, "w2r": w2r}
        for i in range(NCORES)
    ]
    res = run_bass_kernel_spmd(
        nc, in_maps, core_ids=list(range(NCORES)), trace=trace, **run_kwargs
    )
    _CACHE["last_results"] = res
    out = np.concatenate([res.results[i]["out"] for i in range(NCORES)], axis=0)
    return out


# revision 21
# speedup vs baseline: 3.4884x; 1.0773x over previous
"""Trainium2 Bass kernel for nn_DynamicSparseConv.

Model (per sample):
    y  = mean(x, HW)                        [C]
    h  = gelu(y @ w1.T)                     [MID]
    w  = softmax((h @ w2.T).reshape(C, 9))  per-channel 3x3 kernels
    out = depthwise3x3(x, w) + x

Sharding: pure data parallel, batch 32 -> 4 samples on each of 8 cores.

Per-core design (per (sample b, channel-block cb of 128)):
  - x tile kept resident in SBUF as [128, 66, 64] (rows padded with zeros),
    read once from HBM, output written once: memory roofline ~32MiB/core.
  - channel means via DVE reduce over the interior view.
  - tiny MLP on PE (K=128 / K=32 matmuls, N=1), gelu via tanh formula
    (h_pre is in [-0.04, 0.04] so tanh-gelu == erf-gelu to ~1e-9; tanh and
    exp live in the same ACT table set, avoiding table swaps).
  - softmax over 9 taps: ACT exp with accum_out denominator, DVE reciprocal.
  - conv: 9 taps as diagonal-weighted matmuls on PE accumulating in PSUM.
    The x tile is stored flat as [128, 1 + 66*64 + 1]: rows padded with zero
    rows, plus one zero element on each end.  Every tap (r, s) is then a
    full-width contiguous [128, 512] matmul at flat offset r*64 + s; the
    horizontal wrap-around contributions (out col 0 picking up the previous
    row's col 63 for s=-1 taps, etc.) are subtracted afterwards with a few
    small strided DVE ops on the two edge columns.  Residual (+x) is fused
    into the DVE PSUM->SBUF merge.
"""

import numpy as np
from contextlib import ExitStack

import concourse.bass as bass
import concourse.tile as tile
from concourse import mybir
from concourse._compat import with_exitstack
from concourse.masks import make_identity
from concourse.bass_utils import run_bass_kernel_spmd

F32 = mybir.dt.float32
BF16 = mybir.dt.bfloat16
AL = mybir.AluOpType
AF = mybir.ActivationFunctionType

B, C, H, W = 32, 256, 64, 64
MID = 32
NCORES = 8
BPC = B // NCORES          # samples per core
P = 128
CB = C // P                # channel blocks
HP = H + 2                 # padded rows
FREE = H * W               # 4096
NCHUNK = 8                 # PSUM chunks per tile (8 output rows each)
RPC = H // NCHUNK          # rows per chunk

# tap order: full-coverage tap (0,0) first (it initializes every PSUM element
# of the accumulation group with start=True)
TAPS = [(0, 0), (-1, 0), (1, 0), (0, -1), (0, 1), (-1, -1), (-1, 1), (1, -1), (1, 1)]

SQRT_2_OVER_PI = 0.7978845608028654
GELU_C = 0.044715


# bf16 conv-input tile, flat padded layout:
# [0 pad elem][row -1 zeros][64 x rows][row 64 zeros][0 pad elem];
# logical padded row r (r in [-1, 64]) starts at flat offset 1 + (r+1)*64.
# The f32 x tile stays plain [P, 4096] (residual + exactness of means).
XF = 2 + HP * W            # 4226
INT0 = 1 + W               # interior (x row 0 col 0) flat offset = 65
CH = RPC * W               # 512 elements per PSUM chunk


@with_exitstack
def _build_body(ctx: ExitStack, tc: "tile.TileContext", x, w1t, w2r, out):
    nc = tc.nc

    consts = ctx.enter_context(tc.tile_pool(name="consts", bufs=1))
    xpool = ctx.enter_context(tc.tile_pool(name="xpool", bufs=5))
    xbpool = ctx.enter_context(tc.tile_pool(name="xbpool", bufs=6))
    opool = ctx.enter_context(tc.tile_pool(name="opool", bufs=2))
    mpool = ctx.enter_context(tc.tile_pool(name="mpool", bufs=4))
    dpool = ctx.enter_context(tc.tile_pool(name="dpool", bufs=2 * len(TAPS)))
    cpsum = ctx.enter_context(tc.tile_pool(name="cpsum", bufs=6, space="PSUM"))
    spsum = ctx.enter_context(tc.tile_pool(name="spsum", bufs=2, space="PSUM"))

    st = {}  # per-sample pipeline state

    def load(b):
        """DMA in both channel blocks of sample b + bf16 cast + channel sums."""
        xts, xbs = [], []
        sums = mpool.tile([P, CB], F32, name=f"sums{b}", tag="sums")
        for cb in range(CB):
            xt = xpool.tile([P, FREE], F32, name=f"xt{b}_{cb}", tag="xt")
            nc.sync.dma_start(
                out=xt,
                in_=x[b, cb * P:(cb + 1) * P].rearrange("c h w -> c (h w)"),
            )
            xb = xbpool.tile([P, XF], BF16, name=f"xb{b}_{cb}", tag="xb")
            nc.gpsimd.memset(xb[:, 0:INT0], 0.0)
            nc.gpsimd.memset(xb[:, INT0 + FREE:XF], 0.0)
            # one ACT pass: f32->bf16 cast AND per-channel sums (accum_out)
            nc.scalar.activation(
                out=xb[:, INT0:INT0 + FREE], in_=xt, func=AF.Copy,
                accum_out=sums[:, cb:cb + 1],
            )
            xts.append(xt)
            xbs.append(xb)
        st[b] = {"xts": xts, "xbs": xbs, "sums": sums}

    def prep(b, weights):
        """MLP -> softmax tap weights -> diagonal matrices for sample b."""
        w1t_sb, w2r_sb, ident = weights
        sums = st[b]["sums"]
        sums_bf = mpool.tile([P, CB], BF16, name=f"sums_bf{b}", tag="sums_bf")
        nc.vector.tensor_copy(sums_bf, sums)
        hps = spsum.tile([P, 9], F32, name=f"hps{b}", tag="sps")
        for cb in range(CB):
            nc.tensor.matmul(
                hps[:MID, 0:1], lhsT=w1t_sb[:, cb, :], rhs=sums_bf[:, cb:cb + 1],
                start=(cb == 0), stop=(cb == CB - 1),
            )
        # u = h_pre / (H*W)   (ACT copy with scale, PSUM -> SBUF)
        u = mpool.tile([MID, 1], F32, name=f"u{b}", tag="u")
        nc.scalar.mul(u, hps[:MID, 0:1], 1.0 / FREE)
        # tanh-based gelu: g = 0.5*u*(1+tanh(sqrt(2/pi)*(u + 0.044715 u^3)))
        # (the 0.5 is folded into w2r on the host)
        sq = mpool.tile([MID, 1], F32, name=f"sq{b}", tag="sq")
        nc.vector.tensor_mul(sq, u, u)
        c1 = mpool.tile([MID, 1], F32, name=f"c1{b}", tag="c1")
        nc.vector.tensor_scalar(
            out=c1, in0=sq, scalar1=GELU_C, scalar2=1.0, op0=AL.mult, op1=AL.add,
        )
        arg = mpool.tile([MID, 1], F32, name=f"arg{b}", tag="arg")
        nc.vector.tensor_mul(arg, u, c1)
        th = mpool.tile([MID, 1], F32, name=f"th{b}", tag="th")
        nc.scalar.activation(th, arg, AF.Tanh, scale=SQRT_2_OVER_PI)
        t1 = mpool.tile([MID, 1], F32, name=f"t1{b}", tag="t1")
        nc.vector.tensor_scalar_add(t1, th, 1.0)
        g = mpool.tile([MID, 1], BF16, name=f"g{b}", tag="g")
        nc.vector.tensor_mul(g, u, t1)

        st[b]["smw"] = []
        st[b]["diags"] = []
        for cb in range(CB):
            wg = spsum.tile([P, 9], F32, name=f"wg{b}_{cb}", tag="sps")
            for t in range(9):
                nc.tensor.matmul(
                    wg[:, t:t + 1], lhsT=w2r_sb[:, cb * 9 + t, :], rhs=g,
                    start=True, stop=True,
                )
            ew = mpool.tile([P, 9], F32, name=f"ew{b}_{cb}", tag="ew")
            den = mpool.tile([P, 1], F32, name=f"den{b}_{cb}", tag="den")
            nc.scalar.activation(ew, wg, AF.Exp, accum_out=den)
            rden = mpool.tile([P, 1], F32, name=f"rden{b}_{cb}", tag="rden")
            nc.vector.reciprocal(rden, den)
            smw = mpool.tile([P, 9], F32, name=f"smw{b}_{cb}", tag="smw")
            nc.vector.tensor_scalar_mul(smw, ew, rden)

            diags = {}
            for (r, s) in TAPS:
                tcol = (r + 1) * 3 + (s + 1)
                dg = dpool.tile([P, P], BF16, name=f"dg{b}_{cb}_{tcol}", tag="dg")
                nc.vector.tensor_scalar_mul(dg, ident, smw[:, tcol:tcol + 1])
                diags[(r, s)] = dg
            st[b]["smw"].append(smw)
            st[b]["diags"].append(diags)

    def conv(b):
        """Depthwise conv + residual merge + edge fixups + output DMA."""
        for cb in range(CB):
            xt = st[b]["xts"][cb]
            xb = st[b]["xbs"][cb]
            smw = st[b]["smw"][cb]
            diags = st[b]["diags"][cb]
            ot = opool.tile([P, FREE], F32, name=f"ot{b}_{cb}", tag="ot")
            for q in range(NCHUNK):
                ps = cpsum.tile([P, CH], F32, name=f"ps{b}_{cb}_{q}", tag="ps")
                for i, (r, s) in enumerate(TAPS):
                    off = INT0 + q * CH + r * W + s
                    nc.tensor.matmul(
                        ps,
                        lhsT=diags[(r, s)],
                        rhs=xb[:, off:off + CH],
                        start=(i == 0), stop=(i == len(TAPS) - 1),
                    )
                # merge + residual: out = psum + x (f32)
                nc.vector.tensor_add(
                    out=ot[:, q * CH:(q + 1) * CH],
                    in0=ps,
                    in1=xt[:, q * CH:(q + 1) * CH],
                )

            # Subtract the horizontal wrap-around garbage on the two edge
            # columns (strided views, DVE), then DMA out — both done per
            # half-tile so the first 1 MiB ships while chunks 4-7 still merge.
            otr = ot.rearrange("p (h w) -> p h w", w=W)
            HH = H // 2
            # out col 0: s=-1 taps read x[c, i-1+r, 63] at flat 64*(i+r+1)
            # out col 63: s=+1 taps read x[c, i+1+r, 0] at flat 64*(i+r+2)+1
            for half in range(2):
                h0 = half * HH
                for col, s, base in ((0, -1, lambda r: W * (r + 1)),
                                     (W - 1, 1, lambda r: W * (r + 2) + 1)):
                    tmp = mpool.tile(
                        [P, HH], F32, name=f"ec{b}_{cb}_{col}_{half}", tag="ec")
                    for k, r in enumerate((-1, 0, 1)):
                        tcol = (r + 1) * 3 + (s + 1)
                        xv = bass.AP(
                            tensor=xb.tensor,
                            offset=xb.offset + base(r) + h0 * W,
                            ap=[list(xb.ap[0]), [W, HH]],
                        )
                        if k == 0:
                            nc.vector.tensor_scalar_mul(
                                tmp, xv, smw[:, tcol:tcol + 1])
                        else:
                            nc.vector.scalar_tensor_tensor(
                                tmp, xv, smw[:, tcol:tcol + 1], tmp,
                                op0=AL.mult, op1=AL.add,
                            )
                    nc.vector.tensor_sub(
                        otr[:, h0:h0 + HH, col], otr[:, h0:h0 + HH, col], tmp)

                # output DMA on the scalar HWDGE queue (parallel issue with
                # the input DMAs on the sync queue)
                nc.scalar.dma_start(
                    out=out[b, cb * P:(cb + 1) * P, h0:h0 + HH].rearrange(
                        "c h w -> c (h w)"),
                    in_=ot[:, h0 * W:(h0 + HH) * W],
                )
        del st[b]

    # ---- emission: sample 0's x DMAs first (they gate the pipeline) -------
    load(0)

    # weights: two fused DMAs + identity
    ident = consts.tile([P, P], F32)
    make_identity(nc, ident)
    w1t_sb = consts.tile([P, CB, MID], BF16)
    nc.sync.dma_start(out=w1t_sb, in_=w1t.rearrange("(cb c) m -> c cb m", cb=CB))
    w2r_sb = consts.tile([MID, CB * 9, P], BF16)
    nc.sync.dma_start(out=w2r_sb, in_=w2r.rearrange("cb t m c -> m (cb t) c"))
    weights = (w1t_sb, w2r_sb, ident)

    load(1)
    prep(0, weights)
    for b in range(BPC):
        if b + 2 < BPC:
            load(b + 2)
        if b + 1 < BPC:
            prep(b + 1, weights)
        conv(b)


def build_nc():
    nc = bass.Bass(trn_type="TRN2")
    x = nc.dram_tensor("x", [BPC, C, H, W], F32, kind="ExternalInput")
    w1t = nc.dram_tensor("w1t", [C, MID], BF16, kind="ExternalInput")
    w2r = nc.dram_tensor("w2r", [CB, 9, MID, P], BF16, kind="ExternalInput")
    out = nc.dram_tensor("out", [BPC, C, H, W], F32, kind="ExternalOutput")
    with tile.TileContext(nc) as tc:
        _build_body(tc, x, w1t, w2r, out)
    return nc


def host_prep(w1: np.ndarray, w2: np.ndarray):
    """Layout/dtype-only prep of the (tiny) shared weights."""
    import ml_dtypes

    w1t = np.ascontiguousarray(np.asarray(w1, dtype=np.float32).T)  # [C, MID]
    # w2 rows are r = c*9 + t ; -> [cb, t, mid, c_local], pre-scaled by 0.5
    # (folds the 0.5 of gelu: g_kernel = u*(1+tanh(...)) = 2*gelu(u))
    w2r = np.asarray(w2, dtype=np.float32).reshape(CB, P, 9, MID)
    w2r = np.ascontiguousarray(w2r.transpose(0, 2, 3, 1)) * 0.5
    return w1t.astype(ml_dtypes.bfloat16), w2r.astype(ml_dtypes.bfloat16)


# TPB instructions have a single EVENTS (wait) slot and this walrus refuses
# >1 sync-wait on them (Matmult, TensorScalarPtr, DMACopy, ...).  Drain is
# Tile's standard multi-wait tail barrier, which walrus does handle.
_SPLIT_WAIT_SKIP = {"EventSemaphore"}


def _split_matmul_waits_json(data: bytes) -> bytes:
    """Move excess sync-waits on single-wait-slot instructions onto
    EventSemaphore instructions inserted immediately before them on the same
    engine queue (semantically identical)."""
    import orjson

    m = orjson.loads(data)
    cnt = 0
    for fn in m.get("functions", []):
        for bb in fn.get("blocks", []):
            insts = bb.get("instructions")
            if not insts:
                continue
            out = []
            changed = False
            for ins in insts:
                si = ins.get("sync_info")
                if (
                    ins.get("opcode") not in _SPLIT_WAIT_SKIP
                    and si
                    and len(si.get("on_wait") or []) > 1
                ):
                    waits = si["on_wait"]
                    for w in waits[:-1]:
                        out.append({
                            "name": f"EVW-{cnt}",
                            "opcode": "EventSemaphore",
                            "engine": ins["engine"],
                            "ins": [],
                            "outs": [],
                            "debug": ins.get("debug", 0),
                            "sync_info": {"on_wait": [w], "on_update": []},
                        })
                        cnt += 1
                    si["on_wait"] = [waits[-1]]
                    changed = True
                out.append(ins)
            if changed:
                bb["instructions"] = out
    return orjson.dumps(m)


_CACHE: dict = {}


def _get_nc():
    if "nc" not in _CACHE:
        nc = build_nc()
        orig = nc.to_json_bytes
        nc.to_json_bytes = lambda: _split_matmul_waits_json(orig())
        _CACHE["nc"] = nc
    return _CACHE["nc"]


def kernel(x, w1, w2, trace: bool = False, **run_kwargs):
    x = np.ascontiguousarray(np.asarray(x, dtype=np.float32))
    assert x.shape == (B, C, H, W)
    w1t, w2r = host_prep(w1, w2)

    nc = _get_nc()
    in_maps = [
        {"x": x[i * BPC:(i + 1) * BPC], "w1t": w1t, "w2r": w2r}
        for i in range(NCORES)
    ]
    res = run_bass_kernel_spmd(
        nc, in_maps, core_ids=list(range(NCORES)), trace=trace, **run_kwargs
    )
    _CACHE["last_results"] = res
    out = np.concatenate([res.results[i]["out"] for i in range(NCORES)], axis=0)
    return out


# revision 24
# speedup vs baseline: 3.5240x; 1.0102x over previous
"""Trainium2 Bass kernel for nn_DynamicSparseConv.

Model (per sample):
    y  = mean(x, HW)                        [C]
    h  = gelu(y @ w1.T)                     [MID]
    w  = softmax((h @ w2.T).reshape(C, 9))  per-channel 3x3 kernels
    out = depthwise3x3(x, w) + x

Sharding: pure data parallel, batch 32 -> 4 samples on each of 8 cores.

Per-core design (per (sample b, channel-block cb of 128)):
  - x tile kept resident in SBUF as [128, 66, 64] (rows padded with zeros),
    read once from HBM, output written once: memory roofline ~32MiB/core.
  - channel means via DVE reduce over the interior view.
  - tiny MLP on PE (K=128 / K=32 matmuls, N=1), gelu via tanh formula
    (h_pre is in [-0.04, 0.04] so tanh-gelu == erf-gelu to ~1e-9; tanh and
    exp live in the same ACT table set, avoiding table swaps).
  - softmax over 9 taps: ACT exp with accum_out denominator, DVE reciprocal.
  - conv: 9 taps as diagonal-weighted matmuls on PE accumulating in PSUM.
    The x tile is stored flat as [128, 1 + 66*64 + 1]: rows padded with zero
    rows, plus one zero element on each end.  Every tap (r, s) is then a
    full-width contiguous [128, 512] matmul at flat offset r*64 + s; the
    horizontal wrap-around contributions (out col 0 picking up the previous
    row's col 63 for s=-1 taps, etc.) are subtracted afterwards with a few
    small strided DVE ops on the two edge columns.  Residual (+x) is fused
    into the DVE PSUM->SBUF merge.
"""

import numpy as np
from contextlib import ExitStack

import concourse.bass as bass
import concourse.tile as tile
from concourse import mybir
from concourse._compat import with_exitstack
from concourse.masks import make_identity
from concourse.bass_utils import run_bass_kernel_spmd

F32 = mybir.dt.float32
BF16 = mybir.dt.bfloat16
AL = mybir.AluOpType
AF = mybir.ActivationFunctionType

B, C, H, W = 32, 256, 64, 64
MID = 32
NCORES = 8
BPC = B // NCORES          # samples per core
P = 128
CB = C // P                # channel blocks
HP = H + 2                 # padded rows
FREE = H * W               # 4096
NCHUNK = 8                 # PSUM chunks per tile (8 output rows each)
RPC = H // NCHUNK          # rows per chunk

# tap order: full-coverage tap (0,0) first (it initializes every PSUM element
# of the accumulation group with start=True)
TAPS = [(0, 0), (-1, 0), (1, 0), (0, -1), (0, 1), (-1, -1), (-1, 1), (1, -1), (1, 1)]

SQRT_2_OVER_PI = 0.7978845608028654
GELU_C = 0.044715


# bf16 conv-input tile, flat padded layout:
# [0 pad elem][row -1 zeros][64 x rows][row 64 zeros][0 pad elem];
# logical padded row r (r in [-1, 64]) starts at flat offset 1 + (r+1)*64.
# The f32 x tile stays plain [P, 4096] (residual + exactness of means).
XF = 2 + HP * W            # 4226
INT0 = 1 + W               # interior (x row 0 col 0) flat offset = 65
CH = RPC * W               # 512 elements per PSUM chunk


@with_exitstack
def _build_body(ctx: ExitStack, tc: "tile.TileContext", x, w1t, w2r, out):
    nc = tc.nc

    consts = ctx.enter_context(tc.tile_pool(name="consts", bufs=1))
    xpool = ctx.enter_context(tc.tile_pool(name="xpool", bufs=5))
    xbpool = ctx.enter_context(tc.tile_pool(name="xbpool", bufs=6))
    opool = ctx.enter_context(tc.tile_pool(name="opool", bufs=2))
    mpool = ctx.enter_context(tc.tile_pool(name="mpool", bufs=4))
    dpool = ctx.enter_context(tc.tile_pool(name="dpool", bufs=2 * len(TAPS)))
    cpsum = ctx.enter_context(tc.tile_pool(name="cpsum", bufs=6, space="PSUM"))
    spsum = ctx.enter_context(tc.tile_pool(name="spsum", bufs=2, space="PSUM"))

    st = {}  # per-sample pipeline state

    def load(b, nsplit=1):
        """DMA in both channel blocks of sample b + bf16 cast + channel sums.

        nsplit>1 pipelines DMA and cast at sub-tile granularity (used for the
        first sample to shorten the startup ramp); partial sums land in
        separate columns and the h-matmul accumulates over all of them."""
        xts, xbs = [], []
        sums = mpool.tile([P, CB * nsplit], F32, name=f"sums{b}", tag="sums")
        seg = FREE // nsplit
        for cb in range(CB):
            xt = xpool.tile([P, FREE], F32, name=f"xt{b}_{cb}", tag="xt")
            xb = xbpool.tile([P, XF], BF16, name=f"xb{b}_{cb}", tag="xb")
            nc.gpsimd.memset(xb[:, 0:INT0], 0.0)
            nc.gpsimd.memset(xb[:, INT0 + FREE:XF], 0.0)
            xsrc = x[b, cb * P:(cb + 1) * P].rearrange("c h w -> c (h w)")
            for j in range(nsplit):
                sl = slice(j * seg, (j + 1) * seg)
                nc.sync.dma_start(out=xt[:, sl], in_=xsrc[:, sl])
                # one ACT pass: f32->bf16 cast AND partial channel sums
                nc.scalar.activation(
                    out=xb[:, INT0 + j * seg:INT0 + (j + 1) * seg],
                    in_=xt[:, sl], func=AF.Copy,
                    accum_out=sums[:, cb * nsplit + j:cb * nsplit + j + 1],
                )
            xts.append(xt)
            xbs.append(xb)
        st[b] = {"xts": xts, "xbs": xbs, "sums": sums, "nsplit": nsplit}

    def prep(b, weights):
        """MLP -> softmax tap weights -> diagonal matrices for sample b."""
        w1t_sb, w2r_sb, ident = weights
        sums = st[b]["sums"]
        nsplit = st[b]["nsplit"]
        ncols = CB * nsplit
        sums_bf = mpool.tile([P, ncols], BF16, name=f"sums_bf{b}", tag="sums_bf")
        nc.vector.tensor_copy(sums_bf, sums)
        hps = spsum.tile([P, 9], F32, name=f"hps{b}", tag="sps")
        for j in range(ncols):
            nc.tensor.matmul(
                hps[:MID, 0:1], lhsT=w1t_sb[:, j // nsplit, :],
                rhs=sums_bf[:, j:j + 1],
                start=(j == 0), stop=(j == ncols - 1),
            )
        # u = h_pre / (H*W)   (ACT copy with scale, PSUM -> SBUF)
        u = mpool.tile([MID, 1], F32, name=f"u{b}", tag="u")
        nc.scalar.mul(u, hps[:MID, 0:1], 1.0 / FREE)
        # tanh-based gelu: g = 0.5*u*(1+tanh(sqrt(2/pi)*(u + 0.044715 u^3)))
        # (the 0.5 is folded into w2r on the host)
        sq = mpool.tile([MID, 1], F32, name=f"sq{b}", tag="sq")
        nc.vector.tensor_mul(sq, u, u)
        c1 = mpool.tile([MID, 1], F32, name=f"c1{b}", tag="c1")
        nc.vector.tensor_scalar(
            out=c1, in0=sq, scalar1=GELU_C, scalar2=1.0, op0=AL.mult, op1=AL.add,
        )
        arg = mpool.tile([MID, 1], F32, name=f"arg{b}", tag="arg")
        nc.vector.tensor_mul(arg, u, c1)
        th = mpool.tile([MID, 1], F32, name=f"th{b}", tag="th")
        nc.scalar.activation(th, arg, AF.Tanh, scale=SQRT_2_OVER_PI)
        t1 = mpool.tile([MID, 1], F32, name=f"t1{b}", tag="t1")
        nc.vector.tensor_scalar_add(t1, th, 1.0)
        g = mpool.tile([MID, 1], BF16, name=f"g{b}", tag="g")
        nc.vector.tensor_mul(g, u, t1)

        st[b]["smw"] = []
        st[b]["diags"] = []
        for cb in range(CB):
            wg = spsum.tile([P, 9], F32, name=f"wg{b}_{cb}", tag="sps")
            for t in range(9):
                nc.tensor.matmul(
                    wg[:, t:t + 1], lhsT=w2r_sb[:, cb * 9 + t, :], rhs=g,
                    start=True, stop=True,
                )
            ew = mpool.tile([P, 9], F32, name=f"ew{b}_{cb}", tag="ew")
            den = mpool.tile([P, 1], F32, name=f"den{b}_{cb}", tag="den")
            nc.scalar.activation(ew, wg, AF.Exp, accum_out=den)
            rden = mpool.tile([P, 1], F32, name=f"rden{b}_{cb}", tag="rden")
            nc.vector.reciprocal(rden, den)
            smw = mpool.tile([P, 9], F32, name=f"smw{b}_{cb}", tag="smw")
            nc.vector.tensor_scalar_mul(smw, ew, rden)

            diags = {}
            for (r, s) in TAPS:
                tcol = (r + 1) * 3 + (s + 1)
                dg = dpool.tile([P, P], BF16, name=f"dg{b}_{cb}_{tcol}", tag="dg")
                nc.vector.tensor_scalar_mul(dg, ident, smw[:, tcol:tcol + 1])
                diags[(r, s)] = dg
            st[b]["smw"].append(smw)
            st[b]["diags"].append(diags)

    def conv(b):
        """Depthwise conv + residual merge + edge fixups + output DMA."""
        for cb in range(CB):
            xt = st[b]["xts"][cb]
            xb = st[b]["xbs"][cb]
            smw = st[b]["smw"][cb]
            diags = st[b]["diags"][cb]
            ot = opool.tile([P, FREE], F32, name=f"ot{b}_{cb}", tag="ot")
            for q in range(NCHUNK):
                ps = cpsum.tile([P, CH], F32, name=f"ps{b}_{cb}_{q}", tag="ps")
                for i, (r, s) in enumerate(TAPS):
                    off = INT0 + q * CH + r * W + s
                    nc.tensor.matmul(
                        ps,
                        lhsT=diags[(r, s)],
                        rhs=xb[:, off:off + CH],
                        start=(i == 0), stop=(i == len(TAPS) - 1),
                    )
                # merge + residual: out = psum + x (f32)
                nc.vector.tensor_add(
                    out=ot[:, q * CH:(q + 1) * CH],
                    in0=ps,
                    in1=xt[:, q * CH:(q + 1) * CH],
                )

            # Subtract the horizontal wrap-around garbage on the two edge
            # columns (strided views, DVE), then DMA out — both done per
            # half-tile so the first 1 MiB ships while chunks 4-7 still merge.
            otr = ot.rearrange("p (h w) -> p h w", w=W)
            HH = H // 2
            # out col 0: s=-1 taps read x[c, i-1+r, 63] at flat 64*(i+r+1)
            # out col 63: s=+1 taps read x[c, i+1+r, 0] at flat 64*(i+r+2)+1
            for half in range(2):
                h0 = half * HH
                for col, s, base in ((0, -1, lambda r: W * (r + 1)),
                                     (W - 1, 1, lambda r: W * (r + 2) + 1)):
                    tmp = mpool.tile(
                        [P, HH], F32, name=f"ec{b}_{cb}_{col}_{half}", tag="ec")
                    for k, r in enumerate((-1, 0, 1)):
                        tcol = (r + 1) * 3 + (s + 1)
                        xv = bass.AP(
                            tensor=xb.tensor,
                            offset=xb.offset + base(r) + h0 * W,
                            ap=[list(xb.ap[0]), [W, HH]],
                        )
                        if k == 0:
                            nc.vector.tensor_scalar_mul(
                                tmp, xv, smw[:, tcol:tcol + 1])
                        else:
                            nc.vector.scalar_tensor_tensor(
                                tmp, xv, smw[:, tcol:tcol + 1], tmp,
                                op0=AL.mult, op1=AL.add,
                            )
                    nc.vector.tensor_sub(
                        otr[:, h0:h0 + HH, col], otr[:, h0:h0 + HH, col], tmp)

                # output DMA on the scalar HWDGE queue (parallel issue with
                # the input DMAs on the sync queue)
                nc.scalar.dma_start(
                    out=out[b, cb * P:(cb + 1) * P, h0:h0 + HH].rearrange(
                        "c h w -> c (h w)"),
                    in_=ot[:, h0 * W:(h0 + HH) * W],
                )
        del st[b]

    # ---- emission: sample 0's x DMAs first (they gate the pipeline) -------
    load(0, nsplit=4)

    # weights: two fused DMAs + identity
    ident = consts.tile([P, P], F32)
    make_identity(nc, ident)
    w1t_sb = consts.tile([P, CB, MID], BF16)
    nc.sync.dma_start(out=w1t_sb, in_=w1t.rearrange("(cb c) m -> c cb m", cb=CB))
    w2r_sb = consts.tile([MID, CB * 9, P], BF16)
    nc.sync.dma_start(out=w2r_sb, in_=w2r.rearrange("cb t m c -> m (cb t) c"))
    weights = (w1t_sb, w2r_sb, ident)

    load(1)
    prep(0, weights)
    for b in range(BPC):
        if b + 2 < BPC:
            load(b + 2)
        if b + 1 < BPC:
            prep(b + 1, weights)
        conv(b)


def build_nc():
    nc = bass.Bass(trn_type="TRN2")
    x = nc.dram_tensor("x", [BPC, C, H, W], F32, kind="ExternalInput")
    w1t = nc.dram_tensor("w1t", [C, MID], BF16, kind="ExternalInput")
    w2r = nc.dram_tensor("w2r", [CB, 9, MID, P], BF16, kind="ExternalInput")
    out = nc.dram_tensor("out", [BPC, C, H, W], F32, kind="ExternalOutput")
    with tile.TileContext(nc) as tc:
        _build_body(tc, x, w1t, w2r, out)
    return nc


def host_prep(w1: np.ndarray, w2: np.ndarray):
    """Layout/dtype-only prep of the (tiny) shared weights."""
    import ml_dtypes

    w1t = np.ascontiguousarray(np.asarray(w1, dtype=np.float32).T)  # [C, MID]
    # w2 rows are r = c*9 + t ; -> [cb, t, mid, c_local], pre-scaled by 0.5
    # (folds the 0.5 of gelu: g_kernel = u*(1+tanh(...)) = 2*gelu(u))
    w2r = np.asarray(w2, dtype=np.float32).reshape(CB, P, 9, MID)
    w2r = np.ascontiguousarray(w2r.transpose(0, 2, 3, 1)) * 0.5
    return w1t.astype(ml_dtypes.bfloat16), w2r.astype(ml_dtypes.bfloat16)


# TPB instructions have a single EVENTS (wait) slot and this walrus refuses
# >1 sync-wait on them (Matmult, TensorScalarPtr, DMACopy, ...).  Drain is
# Tile's standard multi-wait tail barrier, which walrus does handle.
_SPLIT_WAIT_SKIP = {"EventSemaphore"}


def _split_matmul_waits_json(data: bytes) -> bytes:
    """Move excess sync-waits on single-wait-slot instructions onto
    EventSemaphore instructions inserted immediately before them on the same
    engine queue (semantically identical)."""
    import orjson

    m = orjson.loads(data)
    cnt = 0
    for fn in m.get("functions", []):
        for bb in fn.get("blocks", []):
            insts = bb.get("instructions")
            if not insts:
                continue
            out = []
            changed = False
            for ins in insts:
                si = ins.get("sync_info")
                if (
                    ins.get("opcode") not in _SPLIT_WAIT_SKIP
                    and si
                    and len(si.get("on_wait") or []) > 1
                ):
                    waits = si["on_wait"]
                    for w in waits[:-1]:
                        out.append({
                            "name": f"EVW-{cnt}",
                            "opcode": "EventSemaphore",
                            "engine": ins["engine"],
                            "ins": [],
                            "outs": [],
                            "debug": ins.get("debug", 0),
                            "sync_info": {"on_wait": [w], "on_update": []},
                        })
                        cnt += 1
                    si["on_wait"] = [waits[-1]]
                    changed = True
                out.append(ins)
            if changed:
                bb["instructions"] = out
    return orjson.dumps(m)


_CACHE: dict = {}


def _get_nc():
    if "nc" not in _CACHE:
        nc = build_nc()
        orig = nc.to_json_bytes
        nc.to_json_bytes = lambda: _split_matmul_waits_json(orig())
        _CACHE["nc"] = nc
    return _CACHE["nc"]


def kernel(x, w1, w2, trace: bool = False, **run_kwargs):
    x = np.ascontiguousarray(np.asarray(x, dtype=np.float32))
    assert x.shape == (B, C, H, W)
    w1t, w2r = host_prep(w1, w2)

    nc = _get_nc()
    in_maps = [
        {"x": x[i * BPC:(i + 1) * BPC], "w1t": w1t, "w2r": w2r}
        for i in range(NCORES)
    ]
    res = run_bass_kernel_spmd(
        nc, in_maps, core_ids=list(range(NCORES)), trace=trace, **run_kwargs
    )
    _CACHE["last_results"] = res
    out = np.concatenate([res.results[i]["out"] for i in range(NCORES)], axis=0)
    return out
